# revision 1
# baseline (speedup 1.0000x reference)
"""Trainium2 Bass kernel for the EnforcedNeuralODE recurrence.

Reference computation (per timestep):
    x_t = fc_w @ concat(x_{t-1}, f_{t-1}) + fc_b
      i.e. x_t = Wx x_{t-1} + Wf f_{t-1} + b
over T-1 = 4095 steps, batch 256, state 64, force 64.
Output: [T, B, 64] = concat([x_0], [x_1..x_{T-1}]).

Strategy: data-parallel batch shard (32 samples/core across 8 cores); on
each core a blocked parallel scan over K=32-step blocks:
  P1: within-block prefixes (odd steps only, unroll-2 chain), batched
      across the chunk's blocks in the matmul free dim:
        h_{2p+1} = Wx^2 h_{2p-1} + (Wx Wf) f_{2p} + Wf f_{2p+1} + (Wx b + b)
  P2: block-boundary scan (128 sequential tiny steps):
        s_{b+1} = Wx^K s_b + h_{K-1}
  P3: combine (embarrassingly parallel):
        even j: X_j = Wx^{j+1} s + Wx h_{j-1} + Wf f_j + b
        odd  j: X_j = Wx^{j+1} s + h_j
Matrix powers/products precomputed on host (f64, cast f32).

Hardware constraints honored:
  - Every matmul uses contraction rows 0..64 (K=65; row 64 is a host-
    provided zeros row for F/H, a ones row for S so the bias rides the
    matmul as lhsT row 64).  Mixing operand partition-halves between
    matmuls that share PSUM partitions crashes the device
    (NRT_EXEC_UNIT_UNRECOVERABLE), so everything stays on one half;
    uniform (rows, 64-col) tiling mode also avoids PE drain thrash.
  - PSUM column tiles (0,0)/(0,64) pack even/odd steps into one
    [128, N] psum tile so the PSUM->SBUF evacuation runs 128 wide.
"""

import numpy as np
from contextlib import ExitStack

NCORES = 8
BATCH, STATE, FDIM, TIMESPAN = 256, 64, 64, 4096

# per-core tiling
BC = BATCH // NCORES        # 32 batch per core
K = 32                      # steps per block
PAIRS = K // 2              # 16
NB = TIMESPAN // K          # 128 blocks (steps padded 4095 -> 4096)
NBC = 8                     # blocks per chunk
CHUNKS = NB // NBC          # 16
N = NBC * BC                # 256 free-dim per step column
F_COLS = PAIRS * 2 * N      # 8192 forcing cols per chunk (both parities)
H_COLS = PAIRS * N          # 4096 prefix cols per chunk
O_COLS = PAIRS * N          # 4096 output cols per chunk (pair-packed)

_NC_CACHE: dict = {}

# matmul operand dtype: "float32r" runs the PE at 1 cycle/row (vs 4 for
# float32) at ~1.5e-4 per-matmul relative error (TF32-like). Outputs and
# PSUM accumulation stay full fp32 either way.
MM_DTYPE = "bfloat16"


def _set_dims(ncores=8, bc=32, k=32, nbc=8, chunks=16):
    """Override problem dims (testing only). Recomputes derived globals."""
    global NCORES, BATCH, BC, K, PAIRS, NB, NBC, CHUNKS, N
    global F_COLS, H_COLS, O_COLS, TIMESPAN
    NCORES, BC, K, NBC, CHUNKS = ncores, bc, k, nbc, chunks
    BATCH = NCORES * BC
    PAIRS = K // 2
    NB = CHUNKS * NBC
    TIMESPAN = NB * K
    N = NBC * BC
    F_COLS = PAIRS * 2 * N
    H_COLS = PAIRS * N
    O_COLS = PAIRS * N


def _build_nc(chunks, nbc, bc, k):
    """Build + compile the per-core Bass module (SPMD: same NEFF all cores)."""
    import concourse.bass as bass  # noqa: F401
    import concourse.tile as tile
    from concourse import bacc, mybir

    pairs = k // 2
    n = nbc * bc
    f_cols = pairs * 2 * n
    h_cols = pairs * n
    o_cols = pairs * n
    nb = chunks * nbc
    f32 = mybir.dt.float32
    mdt = getattr(mybir.dt, MM_DTYPE)
    AF = mybir.ActivationFunctionType

    nc = bacc.Bacc("TRN2", target_bir_lowering=False, debug=False)

    f_dram = nc.dram_tensor("f", [65, chunks * f_cols], mdt, kind="ExternalInput")
    wpow_dram = nc.dram_tensor("wpow", [65, k * 64], mdt, kind="ExternalInput")
    wsml_dram = nc.dram_tensor("wsml", [65, 720], mdt, kind="ExternalInput")
    s0_dram = nc.dram_tensor("s0", [65, (nb + 1) * bc], mdt, kind="ExternalInput")
    zrow_dram = nc.dram_tensor("zrow", [1, h_cols], mdt, kind="ExternalInput")
    out_dram = nc.dram_tensor("out", [128, chunks * o_cols], f32, kind="ExternalOutput")

    with tile.TileContext(nc) as tc, ExitStack() as ctx:
        singles = ctx.enter_context(tc.tile_pool(name="singles", bufs=1))
        fpool = ctx.enter_context(tc.tile_pool(name="fpool", bufs=4))
        hpool = ctx.enter_context(tc.tile_pool(name="hpool", bufs=2))
        opool = ctx.enter_context(tc.tile_pool(name="opool", bufs=3))
        p1ps = ctx.enter_context(tc.tile_pool(name="p1ps", bufs=2, space="PSUM"))
        p3ps = ctx.enter_context(tc.tile_pool(name="p3ps", bufs=4, space="PSUM"))
        p2ps = ctx.enter_context(tc.tile_pool(name="p2ps", bufs=2, space="PSUM"))

        wpow = singles.tile([65, k * 64], mdt)
        nc.sync.dma_start(out=wpow[:], in_=wpow_dram[:])
        wsml = singles.tile([65, 720], mdt)
        nc.sync.dma_start(out=wsml[:], in_=wsml_dram[:])
        # block start states: [65, (nb+1)*bc]; row 64 = ones (bias row),
        # cols 0:bc = x0^T, rest zeros -- all host-provided (f32r memset is
        # invalid ISA)
        s65 = singles.tile([65, (nb + 1) * bc], mdt)
        nc.sync.dma_start(out=s65[:], in_=s0_dram[:])

        # weight slices inside wsml (columns); row 64 zero everywhere.
        # f32r matmuls must write psum col base 0, so P3 extras use M=128
        # zero-padded weights ([Wx|0] etc) to keep one 128-wide psum tile.
        a_wx2 = wsml[:, 0:64]        # (Wx^2)^T            [65, 64]
        a_wxwf = wsml[:, 64:128]     # (Wx Wf)^T           [65, 64]
        a_wf = wsml[:, 192:256]      # Wf^T                [65, 64]
        a_eye = wsml[:, 256:320]     # I                   [65, 64]
        b2_ap = wsml[0:64, 320:321]  # Wx b + b (bias for P1 copies)
        a_wx128 = wsml[:, 336:464]   # [Wx^T | 0]          [65, 128]
        a_wf128 = wsml[:, 464:592]   # [Wf^T | 0]          [65, 128]
        a_eye128 = wsml[:, 592:720]  # [0 | I]             [65, 128]

        fh_cols = f_cols // 2      # forcing cols per half-chunk tile
        fh_pairs = pairs // 2      # pairs per F tile
        os_pairs = pairs // 2      # pairs per out-stage tile
        os_cols = os_pairs * n

        for c in range(chunks):
            ftiles = []
            for fh in range(2):
                ft = fpool.tile([65, fh_cols], mdt, tag="F")
                nc.sync.dma_start(
                    out=ft[:],
                    in_=f_dram[:, c * f_cols + fh * fh_cols : c * f_cols + (fh + 1) * fh_cols],
                )
                ftiles.append(ft)

            htile = hpool.tile([65, h_cols], mdt, tag="H")
            nc.sync.dma_start(out=htile[64:65, :], in_=zrow_dram[:])

            def fslice(p, parity):
                ft = ftiles[p // fh_pairs]
                base = (p % fh_pairs) * 2 * n + parity * n
                return ft[:, base : base + n]

            # ---- P1: within-block odd prefixes (sequential chain) ----
            for p in range(pairs):
                ps = p1ps.tile([64, n], f32)
                nc.tensor.matmul(ps[:], a_wxwf, fslice(p, 0), start=True, stop=False)
                if p > 0:
                    nc.tensor.matmul(
                        ps[:], a_wx2, htile[:, (p - 1) * n : p * n],
                        start=False, stop=False,
                    )
                nc.tensor.matmul(ps[:], a_wf, fslice(p, 1), start=False, stop=True)
                # h = psum + b2   (ScalarE, PSUM->SBUF with per-partition bias)
                nc.scalar.activation(
                    htile[0:64, p * n : (p + 1) * n], ps[:], AF.Identity, bias=b2_ap
                )

            # ---- P2: block-boundary scan for this chunk's blocks ----
            for blk in range(nbc):
                bg = c * nbc + blk
                ps2 = p2ps.tile([64, bc], f32)
                nc.tensor.matmul(
                    ps2[:],
                    wpow[:, (k - 1) * 64 : k * 64],
                    s65[:, bg * bc : (bg + 1) * bc],
                    start=True, stop=False,
                )
                nc.tensor.matmul(
                    ps2[:],
                    a_eye,
                    htile[:, (pairs - 1) * n + blk * bc : (pairs - 1) * n + (blk + 1) * bc],
                    start=False, stop=True,
                )
                nc.scalar.activation(
                    s65[0:64, (bg + 1) * bc : (bg + 2) * bc], ps2[:], AF.Copy
                )

            # ---- P3: combine + write out ----
            scol = s65[:, c * n : (c + 1) * n]
            for ohalf in range(2):
                ostage = opool.tile([128, os_cols], f32, tag="OS")
                for pp in range(os_pairs):
                    p = ohalf * os_pairs + pp
                    j0, j1 = 2 * p, 2 * p + 1
                    px = p3ps.tile([128, n], f32)
                    # both steps' s-terms in one M=128 matmul (wpow cols are
                    # contiguous for the pair); even extras -> psum 0:64,
                    # odd extra -> psum 64:128
                    nc.tensor.matmul(
                        px[:, :], wpow[:, j0 * 64 : (j1 + 1) * 64], scol,
                        start=True, stop=False,
                    )
                    if p > 0:
                        nc.tensor.matmul(
                            px[:, :], a_wx128, htile[:, (p - 1) * n : p * n],
                            start=False, stop=False,
                        )
                    nc.tensor.matmul(
                        px[:, :], a_wf128, fslice(p, 0), start=False, stop=False
                    )
                    nc.tensor.matmul(
                        px[:, :], a_eye128, htile[:, p * n : (p + 1) * n],
                        start=False, stop=True,
                    )
                    nc.vector.tensor_copy(ostage[:, pp * n : (pp + 1) * n], px[:])
                nc.sync.dma_start(
                    out=out_dram[:, c * o_cols + ohalf * os_cols : c * o_cols + (ohalf + 1) * os_cols],
                    in_=ostage[:],
                )

    nc.compile()
    return nc


def _get_nc():
    key = (CHUNKS, NBC, BC, K)
    if key not in _NC_CACHE:
        _NC_CACHE[key] = _build_nc(CHUNKS, NBC, BC, K)
    return _NC_CACHE[key]


def _host_prep(inputs, forcing, fc_w, fc_b):
    """Build per-core input maps (numpy only, untimed)."""
    S = STATE
    fc_w = np.asarray(fc_w, np.float32)
    fc_b = np.asarray(fc_b, np.float32)
    Wx = fc_w[:, :S].astype(np.float64)
    Wf = fc_w[:, S:].astype(np.float64)
    b = fc_b.astype(np.float64)

    wsml = np.zeros((65, 720), np.float32)
    wsml[0:64, 0:64] = (Wx @ Wx).T.astype(np.float32)
    wsml[0:64, 64:128] = (Wx @ Wf).T.astype(np.float32)
    wsml[0:64, 192:256] = Wf.T.astype(np.float32)
    wsml[0:64, 256:320] = np.eye(64, dtype=np.float32)
    wsml[0:64, 320] = (Wx @ b + b).astype(np.float32)
    wsml[0:64, 336:400] = Wx.T.astype(np.float32)   # [Wx|0] left half
    wsml[0:64, 464:528] = Wf.T.astype(np.float32)   # [Wf|0] left half
    wsml[0:64, 656:720] = np.eye(64, dtype=np.float32)  # [0|I] right half

    # wpow: col block j holds (Wx^{j+1})^T; row 64 = b for even j else 0
    wpow = np.zeros((65, K * 64), np.float32)
    P = np.eye(S, dtype=np.float64)
    for j in range(K):
        P = Wx @ P
        wpow[0:64, j * 64 : (j + 1) * 64] = P.T.astype(np.float32)
        if j % 2 == 0:
            wpow[64, j * 64 : (j + 1) * 64] = b.astype(np.float32)

    # forcing: [T-1, B, F] -> pad -> [65, c, p, parity, blk, bcore] cols
    steps = TIMESPAN
    fpad = np.zeros((steps, BATCH, FDIM), np.float32)
    fpad[: TIMESPAN - 1] = np.asarray(forcing, np.float32)
    # t = (c*NBC + blk)*K + 2p + parity
    arr = fpad.reshape(CHUNKS, NBC, PAIRS, 2, BATCH, FDIM)
    arr = arr.transpose(5, 0, 2, 3, 1, 4)  # [feat, c, p, parity, blk, bfull]

    inputs = np.asarray(inputs, np.float32)
    if MM_DTYPE == "bfloat16":
        import ml_dtypes

        mm_np = ml_dtypes.bfloat16
    else:
        mm_np = np.float32
    zrow = np.zeros((1, H_COLS), mm_np)
    km_nb = NB
    wpow = wpow.astype(mm_np)
    wsml = wsml.astype(mm_np)
    in_maps = []
    for core in range(NCORES):
        bs = slice(core * BC, (core + 1) * BC)
        fcore = np.zeros((65, CHUNKS * F_COLS), mm_np)
        fcore[0:64] = (
            np.ascontiguousarray(arr[..., bs]).reshape(64, CHUNKS * F_COLS).astype(mm_np)
        )
        s0 = np.zeros((65, (km_nb + 1) * BC), mm_np)
        s0[64, :] = 1.0
        s0[0:64, 0:BC] = inputs[bs].T.astype(mm_np)
        in_maps.append(
            {"f": fcore, "wpow": wpow, "wsml": wsml, "s0": s0, "zrow": zrow}
        )
    return in_maps


def _host_decode(results, inputs):
    """Per-core out [128, CHUNKS*O_COLS] -> full [T, B, S]."""
    inputs = np.asarray(inputs, np.float32)
    out = np.empty((TIMESPAN, BATCH, STATE), np.float32)
    out[0] = inputs
    for core in range(NCORES):
        o = results[core]["out"].reshape(2, 64, CHUNKS, PAIRS, NBC, BC)
        # [parity, s, c, p, blk, b] -> [c, blk, p, parity, b, s]
        o = o.transpose(2, 4, 3, 0, 5, 1).reshape(TIMESPAN, BC, STATE)
        out[1:, core * BC : (core + 1) * BC] = o[: TIMESPAN - 1]
    return out


def kernel(inputs, forcing, fc_w, fc_b, timespan):
    from concourse.bass_utils import run_bass_kernel_spmd

    timespan = int(timespan)
    assert timespan == TIMESPAN, f"hardcoded for timespan={TIMESPAN}, got {timespan}"
    nc = _get_nc()
    in_maps = _host_prep(inputs, forcing, fc_w, fc_b)
    res = run_bass_kernel_spmd(nc, in_maps, core_ids=list(range(NCORES)))
    return _host_decode(res.results, inputs)


if __name__ == "__main__":
    nc = _get_nc()
    print("built ok")



# revision 2
# speedup vs baseline: 3.7746x; 3.7746x over previous
"""Trainium2 Bass kernel for the EnforcedNeuralODE recurrence.

Reference (per timestep): x_t = Wx x_{t-1} + Wf f_{t-1} + b over T-1=4095
steps, batch 256, state 64, force 64.  Output [T, B, 64].

Algorithm (per core, 32-sample batch shard, all math bf16 / f32 PSUM):
  Bias fold: f'_t = f_t + Wf^{-1} b, so x_t = Wx x_{t-1} + Wf f'_t.
  Blocks of KB=32 steps; NB=128 blocks; chunk = 16 blocks (free dim
  N=512 cols = 16 blocks x 32 batch); 8 chunks in 2 groups of 4.
  Phase1  g31_blk = sum_j Wx^{31-j} Wf f'_j   (block forcing response;
          16 f-pair matmuls per chunk, accumulated in PSUM)
  P2      block start states s_b: superblock (8 blocks) convolution v_S
          (8 matmuls) + 16-hop scan  s_{8S+8} = Wx^256 s_{8S} + v_S
          + 7-step parallel reconstruction of interior entries.
  Phase2  x-chain per block pair-by-pair, two matmuls per pair tile
          [x_odd; x_even] (M=128), chained through the bf16 out staging:
            x_{2p+1} = Wx^2 x_{2p-1} + WxWf f'_{2p} + Wf f'_{2p+1}
            x_{2p}   = Wx   x_{2p-1} + Wf   f'_{2p}
  All matmuls K=128/M=128 (zero-padded lhsT), N=512 free, bf16 operands:
  uniform tile mode, one PE cost = N cycles; LDWEIGHTS overlaps.
  Groups pipeline: f DMA (sync+scalar HWDGE queues, 2MB/16KB-desc) ->
  phase1 -> P2 -> phase2 (group 1 phase1 interleaved into group 0
  phase2 to keep PE dense) -> out DMA (gpsimd SWDGE + sync queues).
"""

import numpy as np
from contextlib import ExitStack

NCORES = 8
BATCH, STATE, FDIM, TIMESPAN = 256, 64, 64, 4096

BC = BATCH // NCORES    # 32 batch per core
KB = 32                 # steps per block
PAIRS = KB // 2         # 16 step-pairs per block
NB = TIMESPAN // KB     # 128 blocks (4095 steps padded to 4096)
NBC = 16                # blocks per chunk
CHUNKS = NB // NBC      # 8
N = NBC * BC            # 512 free cols per (chunk, pair)
NG = 2                  # chunk groups (pipeline halves)
GC = CHUNKS // NG       # 4 chunks per group
SBK = 8                 # blocks per superblock (P2)
NSB = NB // SBK         # 16 superblocks
SBH = NSB // NG         # 8 superblocks per group

F_COLS = NG * PAIRS * GC * N      # 65536 forcing cols (bf16)
O_COLS = NG * (PAIRS // 2) * 2 * GC * N  # 65536 output cols (bf16)
W_COLS = (PAIRS + 2 + 8 + 1) * 128       # 3456 weight cols

_NC_CACHE: dict = {}


def _build_nc():
    import concourse.bass as bass  # noqa: F401
    import concourse.tile as tile
    from concourse import bacc, mybir

    f32 = mybir.dt.float32
    bf16 = mybir.dt.bfloat16
    AF = mybir.ActivationFunctionType

    nc = bacc.Bacc("TRN2", target_bir_lowering=False, debug=False)

    f_dram = nc.dram_tensor("f", [128, F_COLS], bf16, kind="ExternalInput")
    w_dram = nc.dram_tensor("wts", [128, W_COLS], bf16, kind="ExternalInput")
    s0_dram = nc.dram_tensor("s0", [128, BC], bf16, kind="ExternalInput")
    out_dram = nc.dram_tensor("out", [128, O_COLS], bf16, kind="ExternalOutput")

    with tile.TileContext(nc) as tc, ExitStack() as ctx:
        singles = ctx.enter_context(tc.tile_pool(name="singles", bufs=1))
        opool = ctx.enter_context(tc.tile_pool(name="opool", bufs=3))
        psA = ctx.enter_context(tc.tile_pool(name="psA", bufs=4, space="PSUM"))
        psB = ctx.enter_context(tc.tile_pool(name="psB", bufs=4, space="PSUM"))

        fsb = singles.tile([128, F_COLS], bf16)
        wsb = singles.tile([128, W_COLS], bf16)
        s_sb = singles.tile([128, (NB + 1) * BC], bf16)
        g31 = singles.tile([128, NB * BC], bf16)
        vsb = singles.tile([128, NSB * BC], bf16)

        # weight slices
        def L1(p):
            return wsb[:, p * 128 : (p + 1) * 128]

        Lhx = wsb[:, 2048:2176]
        Lf = wsb[:, 2176:2304]

        def Lj(j):
            return wsb[:, 2304 + j * 128 : 2304 + (j + 1) * 128]

        Lrec = Lj(6)          # (Wx^32)^T
        Lscan = wsb[:, 3328:3456]

        def fv(g, p, ci):
            base = ((g * PAIRS + p) * GC + ci) * N
            return fsb[:, base : base + N]

        # ---- input DMAs (weights/s0 on scalar; f split sync/scalar) ----
        nc.scalar.dma_start(out=wsb[:], in_=w_dram[:])
        nc.scalar.dma_start(out=s_sb[:, 0:BC], in_=s0_dram[:])
        FD = 4  # f DMAs per group (4 pairs = 8192 cols = 2MB each)
        for g in range(NG):
            for d in range(FD):
                c0 = (g * PAIRS + d * 4) * GC * N
                c1 = (g * PAIRS + (d + 1) * 4) * GC * N
                eng = nc.sync if d % 2 == 0 else nc.scalar
                eng.dma_start(out=fsb[:, c0:c1], in_=f_dram[:, c0:c1])

        # strided views for P2: [128, 8 superblocks, 32] at offset j
        def g31v(g, j):
            r = g31[:, g * 2048 : (g + 1) * 2048].rearrange(
                "p (s j b) -> p s j b", s=SBH, j=SBK, b=BC
            )
            return r[:, :, j, :]

        def sv(g, k):
            r = s_sb[:, g * 2048 : (g + 1) * 2048].rearrange(
                "p (s j b) -> p s j b", s=SBH, j=SBK, b=BC
            )
            return r[:, :, k, :]

        def phase1_evac(g, ci, acc, eng):
            c = g * GC + ci
            eng_map = {0: nc.scalar, 1: nc.vector}
            e = eng_map[eng]
            if e is nc.scalar:
                e.activation(g31[:, c * N : (c + 1) * N], acc[:], AF.Copy)
            else:
                e.tensor_copy(g31[:, c * N : (c + 1) * N], acc[:])

        # ---- phase1 group 0 (sweep-major: follows f DMA arrival) ----
        accs0 = [psA.tile([128, N], f32, tag="A", name=f"acc0_{ci}") for ci in range(GC)]
        for p in range(PAIRS):
            for ci in range(GC):
                nc.tensor.matmul(
                    accs0[ci][:], L1(p), fv(0, p, ci),
                    start=(p == 0), stop=(p == PAIRS - 1),
                )
        for ci in range(GC):
            phase1_evac(0, ci, accs0[ci], ci % 2)

        # ---- P2 for one group ----
        def p2(g):
            vt = psA.tile([128, N], f32, tag="A", name="vt")
            for j in range(SBK):
                nc.tensor.matmul(
                    vt[:, 0 : SBH * BC], Lj(j), g31v(g, j),
                    start=(j == 0), stop=(j == SBK - 1),
                )
            nc.scalar.activation(
                vsb[:, g * SBH * BC : (g + 1) * SBH * BC], vt[:, 0 : SBH * BC], AF.Copy
            )
            for s in range(SBH):
                S = g * SBH + s
                e_in, e_out = S * SBK, S * SBK + SBK
                pt = psA.tile([128, N], f32, tag="A", name="pt")
                nc.tensor.matmul(
                    pt[:, 0:BC], Lscan,
                    s_sb[:, e_in * BC : (e_in + 1) * BC],
                    start=True, stop=True,
                )
                nc.vector.tensor_add(
                    s_sb[:, e_out * BC : (e_out + 1) * BC],
                    pt[:, 0:BC],
                    vsb[:, S * BC : (S + 1) * BC],
                )
            for k in range(1, SBK):
                rt = psA.tile([128, N], f32, tag="A", name="rt")
                nc.tensor.matmul(
                    rt[:, 0 : SBH * BC], Lrec, sv(g, k - 1), start=True, stop=True
                )
                nc.vector.tensor_add(sv(g, k), rt[:, 0 : SBH * BC], g31v(g, k - 1))

        p2(0)

        # ---- phase2 for group g; during g=0 interleave group-1 phase1 ----
        def phase2(g, interleave):
            il_accs = {}
            ost = [None] * (PAIRS // 2)
            for p in range(PAIRS):
                pp, pe = p // 2, p % 2
                if pe == 0:
                    ost[pp] = opool.tile([128, 2 * GC * N], bf16, tag="ost", name="ost")
                chain = [psB.tile([128, N], f32, tag="B", name=f"ch{ci}") for ci in range(GC)]
                for ci in range(GC):
                    if p == 0:
                        c = g * GC + ci
                        prev = s_sb[:, c * NBC * BC : (c + 1) * NBC * BC]
                    else:
                        ppp, ppe = (p - 1) // 2, (p - 1) % 2
                        base = ppe * GC * N + ci * N
                        prev = ost[ppp][:, base : base + N]
                    nc.tensor.matmul(chain[ci][:], Lhx, prev, start=True, stop=False)
                for ci in range(GC):
                    nc.tensor.matmul(
                        chain[ci][:], Lf, fv(g, p, ci), start=False, stop=True
                    )
                if interleave:
                    # group-1 phase1, chunk-pair-major: sweeps 0-7 chunks
                    # (0,1), sweeps 8-15 chunks (2,3); 2 pairs per sweep
                    cpair = p // 8
                    q0 = (p % 8) * 2
                    for dci in range(2):
                        ci1 = cpair * 2 + dci
                        if q0 == 0:
                            il_accs[ci1] = psA.tile(
                                [128, N], f32, tag="A", name=f"il{ci1}"
                            )
                        for q in (q0, q0 + 1):
                            nc.tensor.matmul(
                                il_accs[ci1][:], L1(q), fv(1, q, ci1),
                                start=(q == 0), stop=(q == PAIRS - 1),
                            )
                    if p % 8 == 7:
                        for dci in range(2):
                            ci1 = cpair * 2 + dci
                            phase1_evac(1, ci1, il_accs[ci1], dci)
                for ci in range(GC):
                    dst = ost[pp][:, pe * GC * N + ci * N : pe * GC * N + (ci + 1) * N]
                    if (p * GC + ci) % 2 == 0:
                        nc.scalar.activation(dst, chain[ci][:], AF.Copy)
                    else:
                        nc.vector.tensor_copy(dst, chain[ci][:])
                if pe == 1:
                    base = (g * (PAIRS // 2) + pp) * 2 * GC * N
                    eng = nc.gpsimd if pp % 2 == 0 else nc.sync
                    eng.dma_start(
                        out=out_dram[:, base : base + 2 * GC * N], in_=ost[pp][:]
                    )

        phase2(0, interleave=True)
        p2(1)
        phase2(1, interleave=False)

    nc.compile()
    return nc


def _get_nc():
    if "nc" not in _NC_CACHE:
        _NC_CACHE["nc"] = _build_nc()
    return _NC_CACHE["nc"]


def _host_prep(inputs, forcing, fc_w, fc_b):
    """Build per-core input maps (numpy only, untimed)."""
    import ml_dtypes

    bf = ml_dtypes.bfloat16
    inputs = np.asarray(inputs, np.float32)
    fc_w = np.asarray(fc_w, np.float32)
    fc_b = np.asarray(fc_b, np.float32)
    Wx = fc_w[:, :STATE].astype(np.float64)
    Wf = fc_w[:, STATE:].astype(np.float64)
    b = fc_b.astype(np.float64)
    c = np.linalg.solve(Wf, b)

    # powers of Wx
    WxP = {}
    P = np.eye(STATE)
    for j in range(33):
        WxP[j] = P
        P = Wx @ P
    W256 = np.linalg.matrix_power(Wx, 256)

    wts = np.zeros((128, W_COLS), np.float32)
    for p in range(PAIRS):
        wts[0:64, p * 128 : p * 128 + 64] = (WxP[31 - 2 * p] @ Wf).T
        wts[64:128, p * 128 : p * 128 + 64] = (WxP[30 - 2 * p] @ Wf).T
    wts[0:64, 2048:2112] = (WxP[2]).T          # Lhx: x_odd <- Wx^2 x
    wts[0:64, 2112:2176] = Wx.T                # Lhx: x_even <- Wx x
    wts[0:64, 2176:2240] = (Wx @ Wf).T         # Lf: x_odd <- WxWf f0
    wts[0:64, 2240:2304] = Wf.T                # Lf: x_even <- Wf f0
    wts[64:128, 2176:2240] = Wf.T              # Lf: x_odd <- Wf f1
    for j in range(8):
        wts[0:64, 2304 + j * 128 : 2304 + j * 128 + 64] = (
            np.linalg.matrix_power(Wx, (7 - j) * KB)
        ).T
    wts[0:64, 3328:3392] = W256.T
    wts = wts.astype(bf)

    # forcing with bias fold, padded to T steps
    fp = np.zeros((TIMESPAN, BATCH, FDIM), np.float32)
    fp[: TIMESPAN - 1] = np.asarray(forcing, np.float32) + c.astype(np.float32)
    fp[TIMESPAN - 1] = c.astype(np.float32)
    # [Bk, pair, parity, batch, feat] -> [g, ci, blk, p, par, b, feat]
    arr = fp.reshape(NG, GC, NBC, PAIRS, 2, BATCH, FDIM)
    # dram[par*64+feat, ((g*16+p)*4+ci)*512 + blk*32 + b]
    arr = arr.transpose(4, 6, 0, 3, 1, 2, 5)  # [par, feat, g, p, ci, blk, b]

    in_maps = []
    for core in range(NCORES):
        bs = slice(core * BC, (core + 1) * BC)
        fcore = np.ascontiguousarray(arr[..., bs]).reshape(128, F_COLS).astype(bf)
        s0 = np.zeros((128, BC), np.float32)
        s0[0:64] = inputs[bs].T
        in_maps.append({"f": fcore, "wts": wts, "s0": s0.astype(bf)})
    return in_maps


def _host_decode(results, inputs):
    """Per-core out [128, O_COLS] bf16 -> full [T, B, S] f32."""
    inputs = np.asarray(inputs, np.float32)
    out = np.empty((TIMESPAN, BATCH, STATE), np.float32)
    out[0] = inputs
    for core in range(NCORES):
        o = np.asarray(results[core]["out"], dtype=np.float32)
        # [par, feat, g, pp, pe, ci, blk, b]; par 0 = x_{2p+1}, 1 = x_{2p}
        o = o.reshape(2, 64, NG, PAIRS // 2, 2, GC, NBC, BC)
        # -> [g, ci, blk, pp, pe, parflip(even first), b, feat]
        o = o[::-1].transpose(2, 5, 6, 3, 4, 0, 7, 1)
        o = o.reshape(TIMESPAN, BC, STATE)
        out[1:, core * BC : (core + 1) * BC] = o[: TIMESPAN - 1]
    return out


def kernel(inputs, forcing, fc_w, fc_b, timespan):
    from concourse.bass_utils import run_bass_kernel_spmd

    timespan = int(timespan)
    assert timespan == TIMESPAN, f"hardcoded for timespan={TIMESPAN}, got {timespan}"
    nc = _get_nc()
    in_maps = _host_prep(inputs, forcing, fc_w, fc_b)
    res = run_bass_kernel_spmd(nc, in_maps, core_ids=list(range(NCORES)))
    return _host_decode(res.results, inputs)


if __name__ == "__main__":
    nc = _get_nc()
    print("built ok")
